# revision 2
# baseline (speedup 1.0000x reference)
"""Trainium2 Bass kernel for nn_NodeProcessor (GNN message passing), v3.

Computation (per reference):
    agg = segment_sum(edge_attr, col=edge_index[1], N)      # [N, 64]
    h = relu(concat([x, agg]) @ W0 + b0)
    h = relu(h @ W1 + b1)
    h = h @ W2 + b2
    out = layernorm(h) * ln_g + ln_b + x

Distribution: destination-sharded edges, no collectives. Nodes are
degree-sorted globally and dealt round-robin across the 8 cores so every
core sees the IDENTICAL per-bucket degree profile (one SPMD program).
Each core owns 12800 nodes in 100 buckets of 128, paired onto the 128
SBUF partitions feature-major (bucket A feats on partitions 0:64, bucket
B on 64:128). Pair degrees are padded up to EVEN; edge slots for a node
are CONTIGUOUS along the free dim (lane-major, slot-minor).

v3 changes vs v2 (137us) / v1 (161us):
  Scatter rebuilt as SLOT-MAJOR chain adds: attr column block j holds
  edge-slot j of ALL pairs with degree > j (pairs sorted desc by degree,
  so each block is a contiguous prefix of the pair axis). Slot 0 is
  DMA'd straight into the persistent aggregate; each further slot is one
  giant 1D-contiguous in-place DVE add per segment. Pure-1D bf16 SBUF
  tensor_tensor measures 0.52 ns/col (DVE 2x packed mode) where the v1
  3D-strided chains ran ~1x and v2's tensor_reduce never exceeded 1x.
  ~60 instructions total, no degree padding beyond the bucket max.
  Tiles consume pairs from the light END of the pair axis, so tile t is
  unblocked once the chain passes slot d_tile(t); heavy pairs finish
  last and feed the last tiles.
  Mean-free LayerNorm (v2): W2 row-centered on host -> h2 zero-mean over
  features; var = sumsq/64 (Act square -> DVE group-reduce -> Act sqrt
  -> DVE fast-reciprocal -> one DVE broadcast apply); h2^T computed
  directly node-major via lhsT=h1s matmuls (no transposes, no PSUM->SBUF
  copy); ln_b folded into the residual x, b2' seeded into PSUM by a K=1
  ones-row matmul.
"""

import numpy as np
import ml_dtypes
from contextlib import ExitStack

from concourse import bacc, mybir
from concourse.tile import TileContext
from concourse.bass_utils import run_bass_kernel_spmd

BF16 = ml_dtypes.bfloat16

N_NODES = 100000
N_EDGES = 1200000
D = 64          # d_node == d_edge
H = 128         # hidden
EPS = 1e-5
NCORES = 8
P = 128
NPC = 12800     # nodes per core (padded)
NB = 100        # buckets of 128 nodes per core
NPAIR = NB // 2
MB = 512        # MLP mega-tile (nodes)
MT = NPC // MB  # 25 mega-tiles per core
RING = 6        # node-major h2 PSUM ring depth (mega-tiles)
SEGB = (0, 18, 34, NPAIR)   # pair-axis segment boundaries
                            # (even: no tile straddles)
LOOK = 8        # scatter chunks emitted this many tiles ahead of deadline
TSPLIT = 0      # tiles below this use the Act-side LN apply

_prog_cache: dict[tuple, object] = {}
last_results = None


def _build_program(d_pair):
    """d_pair: per-pair padded degree, sorted DESC (pair p owns acc cols
    [p*128,(p+1)*128); slot block j covers pairs [0, K_j))."""
    nc = bacc.Bacc("TRN2", target_bir_lowering=False, debug=False,
                   num_devices=NCORES)
    f32 = mybir.dt.float32
    bf16 = mybir.dt.bfloat16
    AF = mybir.ActivationFunctionType
    ALU = mybir.AluOpType
    AX = mybir.AxisListType
    Fp = int(sum(d_pair))
    maxd = int(d_pair[0])
    K = [int(np.count_nonzero(d_pair > j)) for j in range(maxd)]
    pref_slot = np.zeros(maxd, np.int64)
    np.cumsum(K[:-1], out=pref_slot[1:])
    d_tile = [int(d_pair[NPAIR - 2 - 2 * t]) for t in range(MT)]

    attr_d = nc.dram_tensor("attr", [P, Fp * P], bf16, kind="ExternalInput").ap()
    xT_d = nc.dram_tensor("xT", [D, NPC], bf16, kind="ExternalInput").ap()
    xb_d = nc.dram_tensor("xb", [P, NB * D], bf16, kind="ExternalInput").ap()
    wb_d = nc.dram_tensor("wb", [P, 1216], bf16, kind="ExternalInput").ap()
    cf_d = nc.dram_tensor("cf", [P, 4], f32, kind="ExternalInput").ap()
    out_d = nc.dram_tensor("out", [P, NB * D], bf16, kind="ExternalOutput").ap()

    with TileContext(nc) as tc, ExitStack() as ctx:
        const = ctx.enter_context(tc.tile_pool(name="const", bufs=1))
        chp = ctx.enter_context(tc.tile_pool(name="chp", bufs=8))
        xtp = ctx.enter_context(tc.tile_pool(name="xtp", bufs=4))
        xbp = ctx.enter_context(tc.tile_pool(name="xbp", bufs=5))
        h0sp = ctx.enter_context(tc.tile_pool(name="h0sp", bufs=3))
        h1sp = ctx.enter_context(tc.tile_pool(name="h1sp", bufs=3))
        sqp = ctx.enter_context(tc.tile_pool(name="sqp", bufs=3))
        stp = ctx.enter_context(tc.tile_pool(name="stp", bufs=3))
        zp = ctx.enter_context(tc.tile_pool(name="zp", bufs=3))
        y1p = ctx.enter_context(tc.tile_pool(name="y1p", bufs=3))
        outp = ctx.enter_context(tc.tile_pool(name="outp", bufs=4))
        ps_h0 = ctx.enter_context(tc.tile_pool(name="ps_h0", bufs=2, space="PSUM"))
        ps_h1 = ctx.enter_context(tc.tile_pool(name="ps_h1", bufs=2, space="PSUM"))
        ps_nm = ctx.enter_context(tc.tile_pool(name="ps_nm", bufs=1, space="PSUM"))

        wb = const.tile([P, 1216], bf16, tag="wb")
        nc.sync.dma_start(out=wb[:], in_=wb_d[:])
        cf = const.tile([P, 4], f32, tag="cf")
        nc.sync.dma_start(out=cf[:], in_=cf_d[:])
        w0x = wb[0:D, 0:H]
        w0a_t = wb[:, 128:256]      # [W0a; 0]
        w0a_b = wb[:, 256:384]      # [0; W0a]
        w1 = wb[:, 384:512]
        w2p = wb[:, 512:576]        # row-centered W2
        gt = wb[:, 576:832]         # ln_g tiled 4x, all partitions
        ones1 = wb[0:1, 832:960]    # ones row (K=1 lhsT for b2 broadcast)
        b2row = wb[0:1, 960:1216]   # centered b2 tiled 4x (K=1 rhs)
        b0c = cf[:, 0:1]
        b1c = cf[:, 1:2]
        epsc = cf[:, 3:4]

        # persistent feature-major aggregate: pair p -> cols [p*128,(p+1)*128)
        acc = const.tile([P, NPAIR * P], bf16, tag="acc")
        # rotating node-major h2 (RING mega-tiles deep, f32)
        nm = ps_nm.tile([P, RING * 256], f32, tag="nm")

        # ---- scatter: slot-major 1D chain adds, per (slot, segment) ----
        def emit_slot_seg(j, s):
            kj = K[j]
            lo, hi = SEGB[s], min(kj, SEGB[s + 1])
            if hi <= lo:
                return
            cols = (hi - lo) * P
            src = attr_d[:, (int(pref_slot[j]) + lo) * P:
                         (int(pref_slot[j]) + hi) * P]
            if j == 0:
                nc.sync.dma_start(out=acc[:, lo * P:hi * P], in_=src)
            else:
                ch = chp.tile([P, cols], bf16, tag="ch", name=f"ch{j}_{s}")
                nc.sync.dma_start(out=ch[:], in_=src)
                with nc.allow_low_precision(reason="bf16 segment-sum"):
                    nc.vector.tensor_tensor(
                        out=acc[:, lo * P:hi * P],
                        in0=acc[:, lo * P:hi * P],
                        in1=ch[:], op=ALU.add)

        # ---- MLP stages over 512-node mega-tiles ----
        xts, xbs = {}, {}
        h0ss, h1ss, sqs, stats, zs = {}, {}, {}, {}, {}

        def s0(t):
            if t in xts:        # already prefetched
                return
            xt = xtp.tile([D, MB], bf16, tag="xt", name=f"xt{t}")
            nc.sync.dma_start(out=xt[:], in_=xT_d[:, t * MB:(t + 1) * MB])
            xts[t] = xt

        def s1(t):
            h0 = ps_h0.tile([H, MB], f32, tag="h0", name=f"h0_{t}")
            nc.tensor.matmul(out=h0[:], lhsT=w0x, rhs=xts.pop(t)[:],
                             start=True, stop=False)
            pa = NPAIR - 2 - 2 * t        # tile t eats pairs from light end
            accs = acc[:, pa * P:(pa + 2) * P]
            h0v = h0[:].rearrange("p (u v) -> p u v", u=4)
            nc.tensor.matmul(out=h0v[:, 0::2, :], lhsT=w0a_t, rhs=accs,
                             start=False, stop=False, skip_group_check=True)
            nc.tensor.matmul(out=h0v[:, 1::2, :], lhsT=w0a_b, rhs=accs,
                             start=False, stop=True, skip_group_check=True)
            return h0

        def s2(t, h0):
            h0s = h0sp.tile([H, MB], bf16, tag="h0s", name=f"h0s{t}")
            nc.scalar.activation(out=h0s[:], in_=h0[:], func=AF.Relu,
                                 bias=b0c)
            h0ss[t] = h0s

        def s3(t):
            h1 = ps_h1.tile([H, MB], f32, tag="h1", name=f"h1_{t}")
            nc.tensor.matmul(out=h1[:], lhsT=w1, rhs=h0ss.pop(t)[:],
                             start=True, stop=True)
            return h1

        def s4(t, h1):
            h1s = h1sp.tile([H, MB], bf16, tag="h1s", name=f"h1s{t}")
            nc.scalar.activation(out=h1s[:], in_=h1[:], func=AF.Relu, bias=b1c)
            h1ss[t] = h1s

        def s5(t):
            # h2^T directly node-major: nm[node, g*64+f] = b2'[f]
            #   + sum_k h1s[k, node] W2'[k, f]
            o = (t % RING) * 256
            h1s = h1ss.pop(t)
            nc.tensor.matmul(out=nm[:, o:o + 256], lhsT=ones1, rhs=b2row,
                             start=True, stop=False, skip_group_check=True)
            for g in range(4):
                nc.tensor.matmul(out=nm[:, o + g * D:o + (g + 1) * D],
                                 lhsT=h1s[:, g * P:(g + 1) * P], rhs=w2p,
                                 start=False, stop=(g == 3),
                                 skip_group_check=True)

        def s6(t):
            o = (t % RING) * 256
            sq = sqp.tile([P, 256], bf16, tag="sq", name=f"sq{t}")
            nc.scalar.activation(out=sq[:], in_=nm[:, o:o + 256],
                                 func=AF.Square)
            sqs[t] = sq
            xb = xbp.tile([P, 256], bf16, tag="xb", name=f"xb{t}")
            nc.sync.dma_start(out=xb[:], in_=xb_d[:, t * 256:(t + 1) * 256])
            xbs[t] = xb

        def s7(t):
            st = stp.tile([P, 12], f32, tag="st", name=f"st{t}")
            sqv = sqs.pop(t)[:].rearrange("p (g n) -> p g n", g=4)
            nc.vector.tensor_reduce(out=st[:, 0:4], in_=sqv,
                                    axis=AX.X, op=ALU.add)
            stats[t] = st

        def s8(t):
            st = stats[t]
            nc.scalar.activation(out=st[:, 4:8], in_=st[:, 0:4],
                                 func=AF.Sqrt, scale=1.0 / D, bias=epsc)
            nc.vector.reciprocal_approx_fast(out=st[:, 8:12], in_=st[:, 4:8])

        def s9(t):
            o = (t % RING) * 256
            st = stats.pop(t)
            z = zp.tile([P, 256], bf16, tag="z", name=f"z{t}")
            if t < TSPLIT:
                # scatter phase: DVE is saturated with adds; apply on Act
                for g in range(4):
                    nc.scalar.activation(out=z[:, g * D:(g + 1) * D],
                                         in_=nm[:, o + g * D:o + (g + 1) * D],
                                         func=AF.Identity,
                                         scale=st[:, 8 + g:9 + g])
            else:
                rb = st[:, 8:12].rearrange("p (g x) -> p g x", x=1)
                nc.vector.tensor_tensor(
                    out=z[:].rearrange("p (g n) -> p g n", g=4),
                    in0=nm[:, o:o + 256].rearrange("p (g n) -> p g n", g=4),
                    in1=rb.to_broadcast((P, 4, D)), op=ALU.mult)
            zs[t] = z

        def s10(t):
            y1 = y1p.tile([P, 256], bf16, tag="y1", name=f"y1_{t}")
            nc.gpsimd.tensor_tensor(out=y1[:], in0=zs.pop(t)[:], in1=gt,
                                    op=ALU.mult)
            yo = outp.tile([P, 256], bf16, tag="yo", name=f"yo{t}")
            nc.gpsimd.tensor_tensor(out=yo[:], in0=y1[:], in1=xbs.pop(t)[:],
                                    op=ALU.add)
            nc.sync.dma_start(out=out_d[:, t * 256:(t + 1) * 256], in_=yo[:])

        vals = {}
        psfn = {1: s1, 3: s3}
        csfn = {2: s2, 4: s4}
        sfn = {0: s0, 5: s5, 6: s6, 7: s7, 8: s8, 9: s9, 10: s10}

        def run_stage(s, t):
            if not (0 <= t < MT):
                return
            if s in psfn:
                vals[(s + 1, t)] = psfn[s](t)
            elif s in csfn:
                csfn[s](t, vals.pop((s, t)))
            else:
                sfn[s](t)

        NS = 11

        # deadline-ordered emission: chunk (j, seg) is first needed by the
        # tile eating the largest pair in the block; pump chunks LOOK
        # tiles ahead; the chp pool provides runtime backpressure.
        def tile_of(pair):
            return (NPAIR - 2 - (pair - pair % 2)) // 2

        # chain-aware deadlines: a segment's in-place add chain advances
        # ~R slots per tile of wall clock, so early slots must start
        # (chain-length / R) tiles before the tail's consumer.
        Rr = 2.0
        entries = []
        for s in range(len(SEGB) - 1):
            seg = []
            for j in range(maxd):
                hi = min(K[j], SEGB[s + 1])
                if hi <= SEGB[s]:
                    continue
                seg.append((j, float(tile_of(hi - 1))))
            dl = None
            for i in range(len(seg) - 1, -1, -1):
                j, tn = seg[i]
                dl = tn if dl is None else min(tn, dl - 1.0 / Rr)
                seg[i] = (j, dl)
            for j, d in seg:
                entries.append((max(0.0, d), j, s))
        entries.sort()
        ei = 0

        def pump(tfrontier):
            nonlocal ei
            while ei < len(entries) and entries[ei][0] <= tfrontier:
                emit_slot_seg(entries[ei][1], entries[ei][2])
                ei += 1

        for t in range(min(4, MT)):   # xt DMAs ahead of the attr stream
            s0(t)
        pump(LOOK)
        for q in range(MT + NS):
            for s in range(NS):
                run_stage(s, q - s)
            pump(min(q + LOOK, MT - 1))

    nc.compile()
    return nc


def _host_plan(col):
    """Degree-sort nodes, deal across cores, pair buckets desc (pair p
    owns acc cols [p*128,(p+1)*128)); buckets for xT/xb/out are in TILE
    order (tile t = pairs NPAIR-2-2t, NPAIR-1-2t: lightest first)."""
    NPAD = NCORES * NPC
    deg = np.zeros(NPAD, np.int64)
    deg[:N_NODES] = np.bincount(col, minlength=N_NODES)
    order = np.argsort(deg, kind="stable")          # ascending degree
    dsort = deg[order]
    d_blk = dsort.reshape(NB, NCORES * P).max(axis=1)
    d_blk = np.maximum(d_blk, 1).astype(np.int64)
    bo_desc = np.argsort(-d_blk, kind="stable")     # desc degree
    d_new = d_blk[bo_desc]
    d_pair = np.maximum(d_new[0::2], d_new[1::2])   # non-increasing
    inv_bo = np.empty(NB, np.int64)
    inv_bo[bo_desc] = np.arange(NB)                 # old bucket -> desc idx
    # tile-major bucket order for xT/xb/out
    border = np.empty(NB, np.int64)
    for t in range(MT):
        pa = NPAIR - 2 - 2 * t
        border[4 * t:4 * t + 4] = (2 * pa, 2 * pa + 1, 2 * pa + 2,
                                   2 * pa + 3)
    bo_tile = bo_desc[border]
    # slot-major column layout
    maxd = int(d_pair[0])
    K = np.array([int(np.count_nonzero(d_pair > j)) for j in range(maxd)],
                 np.int64)
    pref_slot = np.zeros(maxd, np.int64)
    np.cumsum(K[:-1], out=pref_slot[1:])
    return order, dsort, bo_tile, inv_bo, d_pair, pref_slot


def _host_pack(col, edge_attr, order, dsort, inv_bo, pref_slot, Fp):
    E = col.shape[0]
    NPAD = NCORES * NPC
    pos = np.empty(NPAD, np.int64)
    pos[order] = np.arange(NPAD)
    pe = pos[col]                                    # sorted-pos of each dest
    eorder = np.argsort(pe, kind="stable")
    ps = pe[eorder]
    starts = np.zeros(NPAD, np.int64)
    np.cumsum(dsort[:-1], out=starts[1:])
    j = np.arange(E, dtype=np.int64) - starts[ps]
    c = ps % NCORES
    r = ps // NCORES
    b_old = r // P
    lane = r % P
    k = inv_bo[b_old]                                # desc bucket index
    pair = k // 2
    half = k % 2
    COLS = Fp * P
    # slot-major: col = (pref_slot[j] + pair)*128 + lane
    colp = (pref_slot[j] + pair) * P + lane
    rows = c * COLS + colp
    A = np.zeros((NCORES * COLS, P), BF16)
    av = np.asarray(edge_attr, np.float32)[eorder].astype(BF16)
    m0 = half == 0
    A[rows[m0], 0:D] = av[m0]
    A[rows[~m0], D:P] = av[~m0]
    return np.ascontiguousarray(
        A.reshape(NCORES, COLS, P).transpose(0, 2, 1))


def _host_x(x, ln_b, order, bo):
    NPAD = NCORES * NPC
    xpad = np.zeros((NPAD, D), np.float32)
    xpad[:N_NODES] = np.asarray(x, np.float32)
    # node at (core c, new bucket k, lane) = order[(bo[k]*128+lane)*8 + c]
    r_old = (bo[:, None] * P + np.arange(P)[None, :]).reshape(-1)   # [NPC]
    idx = order[r_old[:, None] * NCORES + np.arange(NCORES)[None, :]]
    xTs, xbs = [], []
    bln = np.asarray(ln_b, np.float32)[None, :]
    for cc in range(NCORES):
        xp = xpad[idx[:, cc]]                        # [NPC, 64]
        xTs.append(np.ascontiguousarray(xp.T).astype(BF16))
        xb = (xp + bln).reshape(NB, P, D).transpose(1, 0, 2).reshape(P, NB * D)
        xbs.append(np.ascontiguousarray(xb).astype(BF16))
    return idx, xTs, xbs


def _host_consts(W0, b0, W1, b1, W2, b2, ln_g):
    wb = np.zeros((P, 1216), np.float32)
    W0 = np.asarray(W0, np.float32)
    wb[0:D, 0:H] = W0[0:D]                           # w0x
    wb[0:D, H:2 * H] = W0[D:2 * D]                   # [W0a; 0]
    wb[D:P, 2 * H:3 * H] = W0[D:2 * D]               # [0; W0a]
    wb[:, 384:512] = np.asarray(W1, np.float32)
    W2 = np.asarray(W2, np.float32)
    W2p = W2 - W2.mean(axis=1, keepdims=True)        # row-centered
    wb[:, 512:576] = W2p
    wb[:, 576:832] = np.broadcast_to(
        np.tile(np.asarray(ln_g, np.float32), 4), (P, 256))
    wb[0, 832:960] = 1.0                             # ones row (lhsT)
    b2 = np.asarray(b2, np.float32)
    b2p = b2 - b2.mean()
    wb[0, 960:1216] = np.tile(b2p, 4)
    cf = np.zeros((P, 4), np.float32)
    cf[:, 0] = np.asarray(b0, np.float32)
    cf[:, 1] = np.asarray(b1, np.float32)
    cf[:, 3] = EPS
    return wb.astype(BF16), cf


def kernel(x, edge_index, edge_attr, W0, b0, W1, b1, W2, b2, ln_g, ln_b):
    global last_results
    col = np.asarray(edge_index[1]).astype(np.int64)
    order, dsort, bo, inv_bo, d_pair, pref_slot = _host_plan(col)
    Fp = int(d_pair.sum())

    key = tuple(int(d) for d in d_pair)
    if key not in _prog_cache:
        _prog_cache[key] = _build_program(d_pair)
    nc = _prog_cache[key]

    A = _host_pack(col, edge_attr, order, dsort, inv_bo, pref_slot, Fp)
    idx, xTs, xbs = _host_x(x, ln_b, order, bo)
    wb, cf = _host_consts(W0, b0, W1, b1, W2, b2, ln_g)

    in_maps = []
    for c in range(NCORES):
        in_maps.append({"attr": A[c], "xT": xTs[c], "xb": xbs[c],
                        "wb": wb, "cf": cf})

    res = run_bass_kernel_spmd(nc, in_maps, core_ids=list(range(NCORES)))
    last_results = res

    out = np.zeros((NCORES * NPC, D), np.float32)
    for c in range(NCORES):
        osw = res.results[c]["out"].astype(np.float32)   # [128, NB*64]
        o3 = osw.reshape(P, NB, D).transpose(1, 0, 2).reshape(NPC, D)
        out[idx[:, c]] = o3
    return np.ascontiguousarray(out[:N_NODES])
